# revision 5
# baseline (speedup 1.0000x reference)
"""Causal MHA kernel for 8 TRN2 NeuronCores.

Problem: x[4,2048,1024], 16 heads, hd=64, causal softmax attention, f32.

Sharding: core c handles batch c%4 and head-half c//4 (8 heads).
Each core computes its 8 heads' attention plus the row-slice of the
output projection; the host sums the two partials per batch (the
all-reduce of the row-parallel W_o split) and adds b_o.

v2: all matmul inputs in bf16 (x, W, Q/K/V, probs, ctx). PSUM stays
f32, so accumulation precision is fine; only operand quantization is
bf16 (~0.3% rel err, gate is 2e-2). This doubles DVE throughput,
halves DMA bytes, and enables FWL fast weight loads on the PE. W_q/k/v
are DMA'd once and stay resident in SBUF (baseline re-fetched them per
token chunk, making phase 1 DMA-bound).

Device-side layout: everything transposed. Host ships x[b].T so the
contraction dim (D) lands on SBUF partitions. Projections produce
Q^T/K^T/V^T [64*heads, t]; scores = K_tile^T . Q chunk in PSUM (S^T
layout: keys on partitions, queries on free dim); exp on ScalarE;
causal mask via a precomputed 0/1 bf16 mask multiply on diagonal tiles
only; ctx^T accumulated with V_ext stationary tiles that carry a
ones-column so PSUM row 64 collects the softmax denominators.
"""

import numpy as np
import ml_dtypes

BF16 = ml_dtypes.bfloat16

B, S, D, H, HD = 4, 2048, 1024, 16, 64
HL = 8            # heads per core
F = HL * HD       # 512 local head features
P = 128
CH = 512          # free-dim chunk for matmuls
NKT = D // P      # 8 contraction tiles for projections
NMT = F // P      # 4 head-pair tiles
NCH = S // CH     # 4 token chunks
NKA = S // P      # 16 attention key tiles

_NC_CACHE = {}


def _build_nc(reps=1):
    from contextlib import ExitStack

    import concourse.bass as bass
    import concourse.tile as tile
    from concourse import bacc, mybir
    from concourse.masks import make_identity

    f32 = mybir.dt.float32
    f32r = mybir.dt.float32r
    bf16 = mybir.dt.bfloat16
    AF = mybir.ActivationFunctionType
    ALU = mybir.AluOpType

    def r(ap):
        return ap.bitcast(f32r)

    nc = bacc.Bacc("TRN2", target_bir_lowering=False)
    xt_d = nc.declare_dram_parameter("xt", [D, S], bf16, isOutput=False)
    wq_d = nc.declare_dram_parameter("wq", [D, F], bf16, isOutput=False)
    wk_d = nc.declare_dram_parameter("wk", [D, F], bf16, isOutput=False)
    wv_d = nc.declare_dram_parameter("wv", [D, F], bf16, isOutput=False)
    wo_d = nc.declare_dram_parameter("wo", [F, D], bf16, isOutput=False)
    out_d = nc.declare_dram_parameter("out", [S, D], f32, isOutput=True)
    w_by_name = {"q": wq_d, "k": wk_d, "v": wv_d}

    with tile.TileContext(nc) as tc, ExitStack() as ctx:
        const_pool = ctx.enter_context(tc.tile_pool(name="const", bufs=1))
        qt_pool = ctx.enter_context(tc.tile_pool(name="qt", bufs=1))
        ve_pool = ctx.enter_context(tc.tile_pool(name="ve", bufs=1))
        wo_pool = ctx.enter_context(tc.tile_pool(name="wo", bufs=1))
        ws_pool = ctx.enter_context(tc.tile_pool(name="ws", bufs=1))

        ident = const_pool.tile([P, P], bf16)
        make_identity(nc, ident[:])
        onesf = const_pool.tile([P, 1], bf16)
        nc.vector.memset(onesf[:], 1.0)
        ones_row = const_pool.tile([1, P], bf16)
        nc.vector.memset(ones_row[:], 1.0)

        QT = [qt_pool.tile([P, S], bf16, name=f"qt{m}", tag=f"qt{m}")
              for m in range(NMT)]
        KT = [qt_pool.tile([P, S], bf16, name=f"kt{m}", tag=f"kt{m}")
              for m in range(NMT)]
        # V_ext: per (head, key-tile) a [128, 65] stationary block; col 64
        # stays 1.0 (single memset; projection copies only touch cols 0..63).
        VE = ve_pool.tile([P, HL * NKA * 65], bf16)
        nc.vector.tensor_copy(
            VE[:].rearrange("p (b c) -> p b c", c=65)[:, :, 64:65],
            onesf[:].broadcast_to([P, HL * NKA, 1]),
        )

        masks = const_pool.tile([P, 4 * CH], bf16)
        nc.vector.memset(masks[:], 1.0)
        for j in range(4):
            nc.gpsimd.affine_select(
                out=masks[:, j * CH : (j + 1) * CH],
                in_=masks[:, j * CH : (j + 1) * CH],
                compare_op=ALU.is_ge,
                fill=0.0,
                base=-j * P,
                pattern=[[1, CH]],
                channel_multiplier=-1,
            )

        WO = wo_pool.tile([P, NMT * D], bf16)
        nc.gpsimd.dma_start(
            WO[:].rearrange("p (f n) -> p f n", f=NMT),
            wo_d[:].rearrange("(f p) n -> p f n", p=P),
        )
        # resident weights: [P, NKT * F] each, partition-major contraction
        WS = {}
        for wname in ("v", "k", "q"):
            ws = ws_pool.tile([P, NKT * F], bf16, name=f"ws_{wname}",
                              tag=f"ws_{wname}")
            nc.gpsimd.dma_start(
                ws[:].rearrange("p (k f) -> p k f", k=NKT),
                w_by_name[wname][:].rearrange("(k p) f -> p k f", p=P),
            )
            WS[wname] = ws

        for _rep in range(reps):
            # ---- Phase 1: projections -------------------------------------
            with tc.tile_pool(name="xt", bufs=2) as xt_pool, \
                 tc.tile_pool(name="vstage", bufs=2) as vs_pool, \
                 tc.tile_pool(name="pp", bufs=2, space="PSUM") as pp_pool, \
                 tc.tile_pool(name="pt", bufs=2, space="PSUM") as pt_pool:
                for tch in range(NCH):
                    xt_t = xt_pool.tile([P, NKT * CH], bf16)
                    nc.gpsimd.dma_start(
                        xt_t[:].rearrange("p (k t) -> p k t", k=NKT),
                        xt_d[:].rearrange("(k p) t -> p k t", p=P)[
                            :, :, tch * CH : (tch + 1) * CH
                        ],
                    )
                    for wname in ("v", "k", "q"):
                        ws = WS[wname]
                        for mt in range(NMT):
                            pp = pp_pool.tile([P, CH], f32)
                            for kt in range(NKT):
                                nc.tensor.matmul(
                                    pp[:],
                                    ws[:, kt * F + mt * P : kt * F + (mt + 1) * P],
                                    xt_t[:, kt * CH : (kt + 1) * CH],
                                    start=(kt == 0),
                                    stop=(kt == NKT - 1),
                                )
                            if wname == "q":
                                nc.vector.tensor_copy(
                                    QT[mt][:, tch * CH : (tch + 1) * CH], pp[:]
                                )
                            elif wname == "k":
                                nc.vector.tensor_copy(
                                    KT[mt][:, tch * CH : (tch + 1) * CH], pp[:]
                                )
                            else:
                                vs = vs_pool.tile([P, CH], bf16)
                                nc.scalar.copy(vs[:], pp[:])
                                for j in range(CH // P):
                                    ka = tch * (CH // P) + j
                                    ptp = pt_pool.tile([P, P], bf16)
                                    nc.tensor.transpose(
                                        ptp[:], vs[:, j * P : (j + 1) * P], ident[:]
                                    )
                                    for hh in range(2):
                                        h = 2 * mt + hh
                                        col = (h * NKA + ka) * 65
                                        nc.scalar.copy(
                                            VE[:, col : col + HD],
                                            ptp[:, hh * HD : (hh + 1) * HD],
                                        )

            # ---- Phase 2+3: attention fused with output projection --------
            with tc.tile_pool(name="ptile", bufs=3) as ptile_pool, \
                 tc.tile_pool(name="ctc", bufs=2) as ctc_pool, \
                 tc.tile_pool(name="rec", bufs=2) as rec_pool, \
                 tc.tile_pool(name="bsb", bufs=2) as bsb_pool, \
                 tc.tile_pool(name="osb", bufs=2) as osb_pool, \
                 tc.tile_pool(name="ps_s", bufs=2, space="PSUM") as ps_s_pool, \
                 tc.tile_pool(name="ps_c", bufs=2, space="PSUM") as ps_c_pool, \
                 tc.tile_pool(name="ps_b", bufs=2, space="PSUM") as ps_b_pool:
                for qc in range(NCH):
                    ctc = [ctc_pool.tile([P, CH], bf16, name=f"ctc{m}", tag=f"ctc{m}")
                           for m in range(NMT)]
                    for h in range(HL):
                        mt = h // 2
                        hrow = (h % 2) * HD
                        nka_q = 4 * qc + 4  # causal: key tiles 0..nka_q-1
                        pc = ps_c_pool.tile([HD + 1, CH], f32, tag="pc")
                        for kt2 in range(0, nka_q, 2):
                            ps2 = ps_s_pool.tile([P, 2 * CH], f32)
                            pt2 = ptile_pool.tile([P, 2 * CH], bf16)
                            for u in range(2):
                                kt = kt2 + u
                                nc.tensor.matmul(
                                    ps2[:, u * CH : (u + 1) * CH],
                                    KT[mt][hrow : hrow + HD,
                                           kt * P : (kt + 1) * P],
                                    QT[mt][hrow : hrow + HD,
                                           qc * CH : (qc + 1) * CH],
                                    start=True,
                                    stop=True,
                                )
                            nc.scalar.activation(
                                pt2[:], ps2[:], AF.Exp, scale=0.125
                            )
                            for u in range(2):
                                kt = kt2 + u
                                if kt >= 4 * qc:  # diagonal tile: mask
                                    j = kt - 4 * qc
                                    nc.vector.tensor_mul(
                                        pt2[:, u * CH : (u + 1) * CH],
                                        pt2[:, u * CH : (u + 1) * CH],
                                        masks[:, j * CH : (j + 1) * CH],
                                    )
                                col = (h * NKA + kt) * 65
                                nc.tensor.matmul(
                                    pc[:],
                                    VE[:, col : col + HD + 1],
                                    pt2[:, u * CH : (u + 1) * CH],
                                    start=(kt == 0),
                                    stop=(kt == nka_q - 1),
                                )
                        rec = rec_pool.tile([1, CH], bf16)
                        with nc.allow_low_precision(
                            reason="1/l rounded to bf16 for PE broadcast"
                        ):
                            nc.vector.reciprocal(rec[:], pc[HD : HD + 1, :])
                        pb = ps_b_pool.tile([HD, CH], f32)
                        nc.tensor.matmul(
                            pb[:], ones_row[:, 0:HD], rec[:],
                            start=True, stop=True
                        )
                        bsb = bsb_pool.tile([HD, CH], bf16)
                        nc.vector.tensor_copy(bsb[:], pb[:])
                        nc.vector.tensor_mul(
                            ctc[mt][hrow : hrow + HD, :],
                            pc[0:HD, :],
                            bsb[:],
                        )
                    # output projection for this token chunk
                    for tt4 in range(CH // P):
                        osb = osb_pool.tile([P, D], f32)
                        for ncol in range(D // CH):
                            po = ps_c_pool.tile([P, CH], f32, tag="pc")
                            for ft in range(NMT):
                                nc.tensor.matmul(
                                    po[:],
                                    ctc[ft][:, tt4 * P : (tt4 + 1) * P],
                                    WO[:, ft * D + ncol * CH
                                       : ft * D + (ncol + 1) * CH],
                                    start=(ft == 0),
                                    stop=(ft == NMT - 1),
                                )
                            if ncol % 2 == 0:
                                nc.vector.tensor_copy(
                                    osb[:, ncol * CH : (ncol + 1) * CH], po[:]
                                )
                            else:
                                nc.scalar.copy(
                                    osb[:, ncol * CH : (ncol + 1) * CH], po[:]
                                )
                        r0 = qc * CH + tt4 * P
                        nc.gpsimd.dma_start(out_d[r0 : r0 + P, :], osb[:])

    nc.compile()
    return nc


def _get_nc(reps=1):
    key = f"nc{reps}"
    if key not in _NC_CACHE:
        _NC_CACHE[key] = _build_nc(reps)
    return _NC_CACHE[key]


def _make_in_maps(inputs):
    x = np.asarray(inputs["x"], dtype=np.float32)
    W_q = np.asarray(inputs["W_q"], dtype=np.float32)
    W_k = np.asarray(inputs["W_k"], dtype=np.float32)
    W_v = np.asarray(inputs["W_v"], dtype=np.float32)
    W_o = np.asarray(inputs["W_o"], dtype=np.float32)
    in_maps = []
    for c in range(8):
        b = c % 4
        hh = c // 4
        cols = slice(hh * F, (hh + 1) * F)
        in_maps.append(
            {
                "xt": np.ascontiguousarray(x[b].T).astype(BF16),
                "wq": np.ascontiguousarray(W_q[:, cols]).astype(BF16),
                "wk": np.ascontiguousarray(W_k[:, cols]).astype(BF16),
                "wv": np.ascontiguousarray(W_v[:, cols]).astype(BF16),
                "wo": np.ascontiguousarray(W_o[cols, :]).astype(BF16),
            }
        )
    return in_maps


def kernel(x, W_q, W_k, W_v, W_o, b_o):
    from concourse.bass_utils import run_bass_kernel_spmd

    b_o = np.asarray(b_o, dtype=np.float32)
    nc = _get_nc()
    in_maps = _make_in_maps(
        {"x": x, "W_q": W_q, "W_k": W_k, "W_v": W_v, "W_o": W_o}
    )
    res = run_bass_kernel_spmd(nc, in_maps, core_ids=list(range(8)))

    full = np.empty((B, S, D), dtype=np.float32)
    for b in range(B):
        full[b] = res.results[b]["out"] + res.results[b + 4]["out"] + b_o
    return full


# revision 31
# speedup vs baseline: 3.7400x; 3.7400x over previous
"""Causal MHA kernel for 8 TRN2 NeuronCores.

Problem: x[4,2048,1024], 16 heads, hd=64, causal softmax attention, f32.

Sharding: core c handles batch c%4 and head-half c//4 (8 heads).
Each core computes its 8 heads' attention plus the row-slice of the
output projection; the host sums the two partials per batch (the
all-reduce of the row-parallel W_o split) and adds b_o.

v5: bf16 operands everywhere (PSUM accumulation stays f32; operand
quantization ~0.5% rel err, gate 2e-2). Weights are DMA'd once and
stay resident in SBUF.

Schedule: one software-pipelined stream over rep x token-chunk. During
attention for query chunk t (paced by ScalarE exp throughput), the PE
work of the NEXT chunk's Q/K/V projections and the PREVIOUS chunk's
output projection is drained in at key-pair granularity, so the
in-order PE queue always has independent work while ScalarE drains.
The chunk stream crosses rep boundaries; Q^T/K^T/V^T live in two
SBUF bank sets (rep parity) so the next rep's projections can overlap
the previous rep's last attention chunk without WAR stalls. ScalarE
runs ONLY exp; every PSUM->SBUF copy is on the Vector engine; memsets
and DMA dispatch are on GpSimd/SP.

Attention uses a flipped ctx product: probs tiles [keys, q] are the
stationary operand and V_ext [keys, hd+1] the moving one, producing
ctx in [q, hd] orientation with full PE output-column utilization
(half the streaming cycles of the [hd+1, q] orientation) and natural
causal skipping of all-masked 128-blocks. The trailing ones-column of
V_ext makes PSUM col 64 of each q-block the softmax denominator, so
normalization is a per-partition reciprocal + scale on DVE (no PE
broadcast). A PE transpose per q-block returns ctx to the f-major
layout the W_o projection needs. Within a head, score matmuls of
key-pair i+1 are emitted before the ctx matmuls of pair i; a head's
ctx transposes are deferred into the next head's stream. Diagonal
128x512 score tiles are trimmed: memset-zero the below-diagonal
columns, exp only the valid range, one [128,128] triangular mask.

Device-side layout: everything transposed. Host ships x[b].T so the
contraction dim (D) lands on SBUF partitions.
"""

import numpy as np
import ml_dtypes

BF16 = ml_dtypes.bfloat16

B, S, D, H, HD = 4, 2048, 1024, 16, 64
HL = 8            # heads per core
F = HL * HD       # 512 local head features
P = 128
CH = 512          # free-dim chunk for matmuls
NKT = D // P      # 8 contraction tiles for projections
NMT = F // P      # 4 head-pair tiles
NCH = S // CH     # 4 token chunks
NKA = S // P      # 16 attention key tiles

_NC_CACHE = {}


def _build_nc(reps=1):
    from contextlib import ExitStack

    import concourse.bass as bass
    import concourse.tile as tile
    from concourse import bacc, mybir
    from concourse.masks import make_identity

    f32 = mybir.dt.float32
    bf16 = mybir.dt.bfloat16
    AF = mybir.ActivationFunctionType
    ALU = mybir.AluOpType

    nc = bacc.Bacc("TRN2", target_bir_lowering=False)
    xt_d = nc.declare_dram_parameter("xt", [D, S], bf16, isOutput=False)
    wq_d = nc.declare_dram_parameter("wq", [D, F], bf16, isOutput=False)
    wk_d = nc.declare_dram_parameter("wk", [D, F], bf16, isOutput=False)
    wv_d = nc.declare_dram_parameter("wv", [D, F], bf16, isOutput=False)
    wo_d = nc.declare_dram_parameter("wo", [F, D], bf16, isOutput=False)
    out_d = nc.declare_dram_parameter("out", [S, D], f32, isOutput=True)
    w_by_name = {"q": wq_d, "k": wk_d, "v": wv_d}

    nbank = 2 if reps > 1 else 1

    with tile.TileContext(nc) as tc, ExitStack() as ctx:
        const_pool = ctx.enter_context(tc.tile_pool(name="const", bufs=1))
        qt_pool = ctx.enter_context(tc.tile_pool(name="qt", bufs=1))
        ve_pool = ctx.enter_context(tc.tile_pool(name="ve", bufs=1))
        wo_pool = ctx.enter_context(tc.tile_pool(name="wo", bufs=1))
        ws_pool = ctx.enter_context(tc.tile_pool(name="ws", bufs=1))
        xt_pool = ctx.enter_context(tc.tile_pool(name="xt", bufs=2))
        vs_pool = ctx.enter_context(tc.tile_pool(name="vstage", bufs=2))
        ptile_pool = ctx.enter_context(tc.tile_pool(name="ptile", bufs=3))
        ctc_pool = ctx.enter_context(tc.tile_pool(name="ctc", bufs=2))
        rec_pool = ctx.enter_context(tc.tile_pool(name="rec", bufs=2))
        ctq_pool = ctx.enter_context(tc.tile_pool(name="ctq", bufs=2))
        osb_pool = ctx.enter_context(tc.tile_pool(name="osb", bufs=2))
        pp_pool = ctx.enter_context(
            tc.tile_pool(name="pp", bufs=2, space="PSUM"))
        ps_s_pool = ctx.enter_context(
            tc.tile_pool(name="ps_s", bufs=2, space="PSUM"))
        pcm_pool = ctx.enter_context(
            tc.tile_pool(name="pcm", bufs=2, space="PSUM"))

        ident = const_pool.tile([P, P], bf16)
        make_identity(nc, ident[:])
        onesf = const_pool.tile([P, 1], bf16)
        nc.vector.memset(onesf[:], 1.0)

        # double-banked Q^T/K^T/V_ext (rep parity) so rep r+1's
        # projections overlap rep r's last attention chunk
        QT, KT, VE4 = [], [], []
        for bk in range(nbank):
            QT.append([qt_pool.tile([P, S], bf16, name=f"qt{bk}_{m}",
                                    tag=f"qt{bk}_{m}")
                       for m in range(NMT)])
            KT.append([qt_pool.tile([P, S], bf16, name=f"kt{bk}_{m}",
                                    tag=f"kt{bk}_{m}")
                       for m in range(NMT)])
            # V_ext: per (head, key-tile) a [128, 65] stationary block;
            # col 64 stays 1.0 (projection copies only touch cols 0..63).
            VE = ve_pool.tile([P, HL * NKA * 65], bf16, name=f"ve{bk}",
                              tag=f"ve{bk}")
            nc.vector.tensor_copy(
                VE[:].rearrange("p (b c) -> p b c", c=65)[:, :, 64:65],
                onesf[:].broadcast_to([P, HL * NKA, 1]),
            )
            VE4.append(VE[:].rearrange("p (h ka c) -> p h ka c",
                                       h=HL, c=65))

        # single [128,128] lower-triangular keep-mask: tri[k,c]=1 iff c>=k
        tri = const_pool.tile([P, P], bf16)
        nc.vector.memset(tri[:], 1.0)
        nc.gpsimd.affine_select(
            out=tri[:],
            in_=tri[:],
            compare_op=ALU.is_ge,
            fill=0.0,
            base=0,
            pattern=[[1, P]],
            channel_multiplier=-1,
        )

        # resident weights, DMA'd in per-128-row slices from the otherwise
        # idle SP sequencer so Pool isn't a dispatch bottleneck.
        WS = {}
        for wname in ("v", "k", "q"):
            ws = ws_pool.tile([P, NKT * F], bf16, name=f"ws_{wname}",
                              tag=f"ws_{wname}")
            for kt in range(NKT):
                nc.sync.dma_start(
                    ws[:, kt * F : (kt + 1) * F],
                    w_by_name[wname][kt * P : (kt + 1) * P, :],
                )
            WS[wname] = ws
        WO = wo_pool.tile([P, NMT * D], bf16)
        for ft in range(NMT):
            nc.sync.dma_start(
                WO[:, ft * D : (ft + 1) * D],
                wo_d[ft * P : (ft + 1) * P, :],
            )

        def dma_xt(tch):
            xt_t = xt_pool.tile([P, NKT * CH], bf16)
            for kt in range(NKT):
                # Pool queue: parallel with SP's weight DMAs
                nc.gpsimd.dma_start(
                    xt_t[:, kt * CH : (kt + 1) * CH],
                    xt_d[kt * P : (kt + 1) * P,
                         tch * CH : (tch + 1) * CH],
                )
            return xt_t

        def proj_half(bk, tch, xt_t, wname, mt, half, state):
            # split per-mt projection into two ~850ns fill units sharing
            # one PSUM accumulator
            ws = WS[wname]
            k0 = half * (NKT // 2)
            if half == 0:
                state["pp"] = pp_pool.tile([P, CH], f32, name="pp",
                                           tag="pp")
            pp = state["pp"]
            for kt in range(k0, k0 + NKT // 2):
                nc.tensor.matmul(
                    pp[:],
                    ws[:, kt * F + mt * P : kt * F + (mt + 1) * P],
                    xt_t[:, kt * CH : (kt + 1) * CH],
                    start=(kt == 0),
                    stop=(kt == NKT - 1),
                )
            if half == 0:
                return
            if wname == "q":
                nc.vector.tensor_copy(
                    QT[bk][mt][:, tch * CH : (tch + 1) * CH], pp[:]
                )
            elif wname == "k":
                nc.vector.tensor_copy(
                    KT[bk][mt][:, tch * CH : (tch + 1) * CH], pp[:]
                )
            else:
                vs = vs_pool.tile([P, CH], bf16)
                nc.vector.tensor_copy(vs[:], pp[:])
                for j in range(CH // P):
                    ka = tch * (CH // P) + j
                    ptp = ps_s_pool.tile([P, P], bf16, tag="ps")
                    nc.tensor.transpose(
                        ptp[:], vs[:, j * P : (j + 1) * P], ident[:]
                    )
                    # both heads' 64-col halves in one copy
                    nc.vector.tensor_copy(
                        VE4[bk][:, 2 * mt : 2 * mt + 2, ka, 0:HD],
                        ptp[:].rearrange(
                            "p (hh c) -> p hh c", hh=2
                        )[:, :, 0:HD],
                    )

        def proj_groups(bk, tch, xt_t):
            out = []
            for w in ("v", "k", "q"):
                for m in range(NMT):
                    st = {}
                    for hf in range(2):
                        out.append(
                            lambda w=w, m=m, hf=hf, st=st:
                                proj_half(bk, tch, xt_t, w, m, hf, st)
                        )
            return out

        def wo_part(qc, ctc, tt4, ncol, state):
            if ncol == 0:
                state["osb"] = osb_pool.tile([P, D], f32, name="osb")
            osb = state["osb"]
            po = pcm_pool.tile([P, CH], f32, tag="pcm")
            for ft in range(NMT):
                nc.tensor.matmul(
                    po[:],
                    ctc[ft][:, tt4 * P : (tt4 + 1) * P],
                    WO[:, ft * D + ncol * CH : ft * D + (ncol + 1) * CH],
                    start=(ft == 0),
                    stop=(ft == NMT - 1),
                )
            nc.vector.tensor_copy(
                osb[:, ncol * CH : (ncol + 1) * CH], po[:]
            )
            if ncol == D // CH - 1:
                r0 = qc * CH + tt4 * P
                nc.gpsimd.dma_start(out_d[r0 : r0 + P, :], osb[:])

        def wo_groups(qc, ctc):
            out = []
            for t in range(CH // P):
                st = {}
                for ncol in range(D // CH):
                    out.append(
                        lambda t=t, ncol=ncol, st=st:
                            wo_part(qc, ctc, t, ncol, st)
                    )
            return out

        def attention_chunk(bk, qc, fill):
            """Attention for query chunk qc reading bank bk, draining
            `fill` (list of emission callables) at key-pair granularity."""
            nka_q = 4 * qc + 4  # causal: key tiles 0..nka_q-1
            total_slots = HL * (nka_q // 2)
            fill_state = [0, 0]  # [next fill idx, slot counter]

            def drain_fill():
                idx, slot = fill_state
                while (idx < len(fill)
                       and idx * total_slots <= slot * len(fill)):
                    fill[idx]()
                    idx += 1
                fill_state[0] = idx
                fill_state[1] = slot + 1

            ctc = [ctc_pool.tile([P, CH], bf16, name=f"ctc{m}",
                                 tag=f"ctc{m}")
                   for m in range(NMT)]

            def emit_norm_dve(pcq, ctq):
                # denominators live at col 64 of each qb block ->
                # per-partition scale, no PE broadcast needed.
                rec4 = rec_pool.tile([P, 4], bf16)
                pcq3 = pcq[:].rearrange("p (qb c) -> p qb c", c=HD + 1)
                rec3 = rec4[:].rearrange("p (b o) -> p b o", o=1)
                with nc.allow_low_precision(
                    reason="1/l rounded to bf16 scale"
                ):
                    nc.vector.reciprocal(rec3, pcq3[:, :, HD : HD + 1])
                nc.vector.tensor_mul(
                    ctq[:].rearrange("p (qb c) -> p qb c", c=HD),
                    pcq3[:, :, 0:HD],
                    rec3.broadcast_to([P, 4, HD]),
                )

            def emit_norm_tpose(ctq, mt, hrow):
                # [q, hd] -> [hd, q] for the f-major Wo projection
                for qb in range(4):
                    ptq = ps_s_pool.tile([HD, P], bf16, tag="ps")
                    nc.tensor.transpose(
                        ptq[:], ctq[:, qb * HD : (qb + 1) * HD], ident[:]
                    )
                    nc.vector.tensor_copy(
                        ctc[mt][hrow : hrow + HD, qb * P : (qb + 1) * P],
                        ptq[:],
                    )

            pending_tpose = None
            for h in range(HL):
                mt = h // 2
                hrow = (h % 2) * HD
                pcq = pcm_pool.tile([P, 4 * (HD + 1)], f32, tag="pcm")
                ctq = ctq_pool.tile([P, 4 * HD], bf16)
                pairs = list(range(0, nka_q, 2))
                ps2s = {}
                pt2s = {}

                def emit_scores(kt2):
                    diag = kt2 >= 4 * qc
                    ps2 = ps_s_pool.tile([P, 2 * CH], f32, tag="ps")
                    pt2 = ptile_pool.tile([P, 2 * CH], bf16)
                    ps2s[kt2] = ps2
                    pt2s[kt2] = pt2
                    for u in range(2):
                        kt = kt2 + u
                        lo = (kt - 4 * qc) * P if diag else 0
                        nc.tensor.matmul(
                            ps2[:, u * CH + lo : (u + 1) * CH],
                            KT[bk][mt][hrow : hrow + HD,
                                       kt * P : (kt + 1) * P],
                            QT[bk][mt][hrow : hrow + HD,
                                       qc * CH + lo : (qc + 1) * CH],
                            start=True,
                            stop=True,
                        )
                    if not diag:
                        nc.scalar.activation(
                            pt2[:], ps2[:], AF.Exp, scale=0.125
                        )
                    else:
                        for u in range(2):
                            lo = (kt2 + u - 4 * qc) * P
                            if lo:
                                nc.gpsimd.memset(
                                    pt2[:, u * CH : u * CH + lo], 0.0
                                )
                            nc.scalar.activation(
                                pt2[:, u * CH + lo : (u + 1) * CH],
                                ps2[:, u * CH + lo : (u + 1) * CH],
                                AF.Exp, scale=0.125,
                            )
                            nc.vector.tensor_mul(
                                pt2[:, u * CH + lo : u * CH + lo + P],
                                pt2[:, u * CH + lo : u * CH + lo + P],
                                tri[:],
                            )

                def emit_ctx(kt2):
                    pt2 = pt2s.pop(kt2)
                    ps2s.pop(kt2)
                    for u in range(2):
                        kt = kt2 + u
                        j = kt - 4 * qc  # diag block index if >= 0
                        for qb in range(max(0, j), 4):
                            # start=True clears has_written for the WHOLE
                            # bank, so only the very first matmul into the
                            # pcq bank may set it; later qb first-writes
                            # overwrite-where-clear per element.
                            nc.tensor.matmul(
                                pcq[:, qb * (HD + 1)
                                    : qb * (HD + 1) + HD + 1],
                                pt2[:, u * CH + qb * P
                                    : u * CH + (qb + 1) * P],
                                VE4[bk][:, h, kt, :],
                                start=(kt == 0 and qb == max(0, j)),
                                stop=(kt == min(nka_q - 1, 4 * qc + qb)),
                            )

                # software-pipelined emission: scores of pair i+1 land on
                # the PE queue before ctx of pair i, so PE streams while
                # ScalarE runs exp; the previous head's ctx transposes
                # are emitted one pair in.
                emit_scores(pairs[0])
                for i, kt2 in enumerate(pairs):
                    if i + 1 < len(pairs):
                        emit_scores(pairs[i + 1])
                    emit_ctx(kt2)
                    if (i == min(1, len(pairs) - 1)
                            and pending_tpose is not None):
                        emit_norm_tpose(*pending_tpose)
                        pending_tpose = None
                    drain_fill()
                # the DVE part runs now (frees pcq for the pool); only
                # the PE transposes are deferred.
                emit_norm_dve(pcq, ctq)
                pending_tpose = (ctq, mt, hrow)
            emit_norm_tpose(*pending_tpose)
            while fill_state[0] < len(fill):
                fill[fill_state[0]]()
                fill_state[0] += 1
            return ctc

        # ---- the flattened rep x chunk stream -------------------------
        seq = [(r, t) for r in range(reps) for t in range(NCH)]
        xt_t = dma_xt(0)
        for g in proj_groups(0, 0, xt_t):
            g()
        pending_wo = []
        for idx, (r, tch) in enumerate(seq):
            fill = list(pending_wo)
            pending_wo = []
            if idx + 1 < len(seq):
                nr, nt = seq[idx + 1]
                xt_t = dma_xt(nt)
                fill += proj_groups(nr % nbank, nt, xt_t)
            ctc = attention_chunk(r % nbank, tch, fill)
            pending_wo = wo_groups(tch, ctc)
        for g in pending_wo:
            g()

    nc.compile()
    return nc


def _get_nc(reps=1):
    key = f"nc{reps}"
    if key not in _NC_CACHE:
        _NC_CACHE[key] = _build_nc(reps)
    return _NC_CACHE[key]


def _make_in_maps(inputs):
    x = np.asarray(inputs["x"], dtype=np.float32)
    W_q = np.asarray(inputs["W_q"], dtype=np.float32)
    W_k = np.asarray(inputs["W_k"], dtype=np.float32)
    W_v = np.asarray(inputs["W_v"], dtype=np.float32)
    W_o = np.asarray(inputs["W_o"], dtype=np.float32)
    in_maps = []
    for c in range(8):
        b = c % 4
        hh = c // 4
        cols = slice(hh * F, (hh + 1) * F)
        in_maps.append(
            {
                "xt": np.ascontiguousarray(x[b].T).astype(BF16),
                "wq": np.ascontiguousarray(W_q[:, cols]).astype(BF16),
                "wk": np.ascontiguousarray(W_k[:, cols]).astype(BF16),
                "wv": np.ascontiguousarray(W_v[:, cols]).astype(BF16),
                "wo": np.ascontiguousarray(W_o[cols, :]).astype(BF16),
            }
        )
    return in_maps


def kernel(x, W_q, W_k, W_v, W_o, b_o):
    from concourse.bass_utils import run_bass_kernel_spmd

    b_o = np.asarray(b_o, dtype=np.float32)
    nc = _get_nc()
    in_maps = _make_in_maps(
        {"x": x, "W_q": W_q, "W_k": W_k, "W_v": W_v, "W_o": W_o}
    )
    res = run_bass_kernel_spmd(nc, in_maps, core_ids=list(range(8)))

    full = np.empty((B, S, D), dtype=np.float32)
    for b in range(B):
        full[b] = res.results[b]["out"] + res.results[b + 4]["out"] + b_o
    return full


# revision 35
# speedup vs baseline: 3.9046x; 1.0440x over previous
"""Causal MHA kernel for 8 TRN2 NeuronCores.

Problem: x[4,2048,1024], 16 heads, hd=64, causal softmax attention, f32.

Sharding: core c handles batch c%4 and head-half c//4 (8 heads).
Each core computes its 8 heads' attention plus the row-slice of the
output projection; the host sums the two partials per batch (the
all-reduce of the row-parallel W_o split) and adds b_o.

v5: bf16 operands everywhere (PSUM accumulation stays f32; operand
quantization ~0.5% rel err, gate 2e-2). Weights are DMA'd once and
stay resident in SBUF.

Schedule: one software-pipelined stream over rep x token-chunk. During
attention for query chunk t (paced by ScalarE exp throughput), the PE
work of the NEXT chunk's Q/K/V projections and the PREVIOUS chunk's
output projection is drained in at key-pair granularity, so the
in-order PE queue always has independent work while ScalarE drains.
The chunk stream crosses rep boundaries; Q^T/K^T/V^T live in two
SBUF bank sets (rep parity) so the next rep's projections can overlap
the previous rep's last attention chunk without WAR stalls. ScalarE
runs ONLY exp; every PSUM->SBUF copy is on the Vector engine; memsets
and DMA dispatch are on GpSimd/SP.

Attention uses a flipped ctx product: probs tiles [keys, q] are the
stationary operand and V_ext [keys, hd+1] the moving one, producing
ctx in [q, hd] orientation with full PE output-column utilization
(half the streaming cycles of the [hd+1, q] orientation) and natural
causal skipping of all-masked 128-blocks. The trailing ones-column of
V_ext makes PSUM col 64 of each q-block the softmax denominator, so
normalization is a per-partition reciprocal + scale on DVE (no PE
broadcast). A PE transpose per q-block returns ctx to the f-major
layout the W_o projection needs. Within a head, score matmuls of
key-pair i+1 are emitted before the ctx matmuls of pair i; a head's
ctx transposes are deferred into the next head's stream. Diagonal
128x512 score tiles are trimmed: memset-zero the below-diagonal
columns, exp only the valid range, one [128,128] triangular mask.

Device-side layout: everything transposed. Host ships x[b].T so the
contraction dim (D) lands on SBUF partitions.
"""

import numpy as np
import ml_dtypes

BF16 = ml_dtypes.bfloat16

B, S, D, H, HD = 4, 2048, 1024, 16, 64
HL = 8            # heads per core
F = HL * HD       # 512 local head features
P = 128
CH = 512          # free-dim chunk for matmuls
NKT = D // P      # 8 contraction tiles for projections
NMT = F // P      # 4 head-pair tiles
NCH = S // CH     # 4 token chunks
NKA = S // P      # 16 attention key tiles

_NC_CACHE = {}


def _build_nc(reps=1):
    from contextlib import ExitStack

    import concourse.bass as bass
    import concourse.tile as tile
    from concourse import bacc, mybir
    from concourse.masks import make_identity

    f32 = mybir.dt.float32
    bf16 = mybir.dt.bfloat16
    AF = mybir.ActivationFunctionType
    ALU = mybir.AluOpType

    nc = bacc.Bacc("TRN2", target_bir_lowering=False)
    xt_d = nc.declare_dram_parameter("xt", [D, S], bf16, isOutput=False)
    wq_d = nc.declare_dram_parameter("wq", [D, F], bf16, isOutput=False)
    wk_d = nc.declare_dram_parameter("wk", [D, F], bf16, isOutput=False)
    wv_d = nc.declare_dram_parameter("wv", [D, F], bf16, isOutput=False)
    wo_d = nc.declare_dram_parameter("wo", [F, D], bf16, isOutput=False)
    out_d = nc.declare_dram_parameter("out", [S, D], f32, isOutput=True)
    w_by_name = {"q": wq_d, "k": wk_d, "v": wv_d}

    nbank = 2 if reps > 1 else 1

    with tile.TileContext(nc) as tc, ExitStack() as ctx:
        const_pool = ctx.enter_context(tc.tile_pool(name="const", bufs=1))
        qt_pool = ctx.enter_context(tc.tile_pool(name="qt", bufs=1))
        ve_pool = ctx.enter_context(tc.tile_pool(name="ve", bufs=1))
        wo_pool = ctx.enter_context(tc.tile_pool(name="wo", bufs=1))
        ws_pool = ctx.enter_context(tc.tile_pool(name="ws", bufs=1))
        xt_pool = ctx.enter_context(tc.tile_pool(name="xt", bufs=2))
        vs_pool = ctx.enter_context(tc.tile_pool(name="vstage", bufs=2))
        ptile_pool = ctx.enter_context(tc.tile_pool(name="ptile", bufs=3))
        ctc_pool = ctx.enter_context(tc.tile_pool(name="ctc", bufs=2))
        rec_pool = ctx.enter_context(tc.tile_pool(name="rec", bufs=2))
        ctq_pool = ctx.enter_context(tc.tile_pool(name="ctq", bufs=2))
        osb_pool = ctx.enter_context(tc.tile_pool(name="osb", bufs=2))
        pp_pool = ctx.enter_context(
            tc.tile_pool(name="pp", bufs=2, space="PSUM"))
        ps_s_pool = ctx.enter_context(
            tc.tile_pool(name="ps_s", bufs=2, space="PSUM"))
        pcm_pool = ctx.enter_context(
            tc.tile_pool(name="pcm", bufs=2, space="PSUM"))

        ident = const_pool.tile([P, P], bf16)
        make_identity(nc, ident[:])
        onesf = const_pool.tile([P, 1], bf16)
        nc.vector.memset(onesf[:], 1.0)

        # double-banked Q^T/K^T/V_ext (rep parity) so rep r+1's
        # projections overlap rep r's last attention chunk
        QT, KT, VE4 = [], [], []
        for bk in range(nbank):
            QT.append([qt_pool.tile([P, S], bf16, name=f"qt{bk}_{m}",
                                    tag=f"qt{bk}_{m}")
                       for m in range(NMT)])
            KT.append([qt_pool.tile([P, S], bf16, name=f"kt{bk}_{m}",
                                    tag=f"kt{bk}_{m}")
                       for m in range(NMT)])
            # V_ext: per (head, key-tile) a [128, 65] stationary block;
            # col 64 stays 1.0 (projection copies only touch cols 0..63).
            VE = ve_pool.tile([P, HL * NKA * 65], bf16, name=f"ve{bk}",
                              tag=f"ve{bk}")
            nc.vector.tensor_copy(
                VE[:].rearrange("p (b c) -> p b c", c=65)[:, :, 64:65],
                onesf[:].broadcast_to([P, HL * NKA, 1]),
            )
            VE4.append(VE[:].rearrange("p (h ka c) -> p h ka c",
                                       h=HL, c=65))

        # single [128,128] lower-triangular keep-mask: tri[k,c]=1 iff c>=k
        tri = const_pool.tile([P, P], bf16)
        nc.vector.memset(tri[:], 1.0)
        nc.gpsimd.affine_select(
            out=tri[:],
            in_=tri[:],
            compare_op=ALU.is_ge,
            fill=0.0,
            base=0,
            pattern=[[1, P]],
            channel_multiplier=-1,
        )

        # resident weights, DMA'd in per-128-row slices from the otherwise
        # idle SP sequencer so Pool isn't a dispatch bottleneck.
        WS = {}
        for wname in ("v", "k", "q"):
            ws = ws_pool.tile([P, NKT * F], bf16, name=f"ws_{wname}",
                              tag=f"ws_{wname}")
            for kt in range(NKT):
                nc.sync.dma_start(
                    ws[:, kt * F : (kt + 1) * F],
                    w_by_name[wname][kt * P : (kt + 1) * P, :],
                )
            WS[wname] = ws
        WO = wo_pool.tile([P, NMT * D], bf16)
        for ft in range(NMT):
            nc.sync.dma_start(
                WO[:, ft * D : (ft + 1) * D],
                wo_d[ft * P : (ft + 1) * P, :],
            )

        def dma_xt(tch):
            xt_t = xt_pool.tile([P, NKT * CH], bf16)
            for kt in range(NKT):
                # Pool queue: parallel with SP's weight DMAs
                nc.gpsimd.dma_start(
                    xt_t[:, kt * CH : (kt + 1) * CH],
                    xt_d[kt * P : (kt + 1) * P,
                         tch * CH : (tch + 1) * CH],
                )
            return xt_t

        def proj_half(bk, tch, xt_t, wname, mt, half, state):
            # split per-mt projection into two ~850ns fill units sharing
            # one PSUM accumulator
            ws = WS[wname]
            k0 = half * (NKT // 2)
            if half == 0:
                state["pp"] = pp_pool.tile([P, CH], f32, name="pp",
                                           tag="pp")
            pp = state["pp"]
            for kt in range(k0, k0 + NKT // 2):
                nc.tensor.matmul(
                    pp[:],
                    ws[:, kt * F + mt * P : kt * F + (mt + 1) * P],
                    xt_t[:, kt * CH : (kt + 1) * CH],
                    start=(kt == 0),
                    stop=(kt == NKT - 1),
                )
            if half == 0:
                return
            if wname == "q":
                nc.vector.tensor_copy(
                    QT[bk][mt][:, tch * CH : (tch + 1) * CH], pp[:]
                )
            elif wname == "k":
                nc.vector.tensor_copy(
                    KT[bk][mt][:, tch * CH : (tch + 1) * CH], pp[:]
                )
            else:
                vs = vs_pool.tile([P, CH], bf16)
                nc.vector.tensor_copy(vs[:], pp[:])
                for j in range(CH // P):
                    ka = tch * (CH // P) + j
                    ptp = ps_s_pool.tile([P, P], bf16, tag="ps")
                    nc.tensor.transpose(
                        ptp[:], vs[:, j * P : (j + 1) * P], ident[:]
                    )
                    # both heads' 64-col halves in one copy
                    nc.vector.tensor_copy(
                        VE4[bk][:, 2 * mt : 2 * mt + 2, ka, 0:HD],
                        ptp[:].rearrange(
                            "p (hh c) -> p hh c", hh=2
                        )[:, :, 0:HD],
                    )

        def proj_groups(bk, tch, xt_t):
            out = []
            for w in ("v", "k", "q"):
                for m in range(NMT):
                    st = {}
                    for hf in range(2):
                        out.append(
                            lambda w=w, m=m, hf=hf, st=st:
                                proj_half(bk, tch, xt_t, w, m, hf, st)
                        )
            return out

        def wo_part(qc, ctc, tt4, ncol, state):
            if ncol == 0:
                state["osb"] = osb_pool.tile([P, D], f32, name="osb")
            osb = state["osb"]
            po = pcm_pool.tile([P, CH], f32, tag="pcm")
            for ft in range(NMT):
                nc.tensor.matmul(
                    po[:],
                    ctc[ft][:, tt4 * P : (tt4 + 1) * P],
                    WO[:, ft * D + ncol * CH : ft * D + (ncol + 1) * CH],
                    start=(ft == 0),
                    stop=(ft == NMT - 1),
                )
            nc.vector.tensor_copy(
                osb[:, ncol * CH : (ncol + 1) * CH], po[:]
            )
            if ncol == D // CH - 1:
                r0 = qc * CH + tt4 * P
                nc.gpsimd.dma_start(out_d[r0 : r0 + P, :], osb[:])

        def wo_groups(qc, ctc):
            out = []
            for t in range(CH // P):
                st = {}
                for ncol in range(D // CH):
                    out.append(
                        lambda t=t, ncol=ncol, st=st:
                            wo_part(qc, ctc, t, ncol, st)
                    )
            return out

        def attention_chunk(bk, qc, fill):
            """Attention for query chunk qc reading bank bk, draining
            `fill` (list of emission callables) at key-pair granularity."""
            nka_q = 4 * qc + 4  # causal: key tiles 0..nka_q-1
            total_slots = HL * (nka_q // 2)
            fill_state = [0, 0]  # [next fill idx, slot counter]

            def drain_fill():
                idx, slot = fill_state
                while (idx < len(fill)
                       and idx * total_slots <= slot * len(fill)):
                    fill[idx]()
                    idx += 1
                fill_state[0] = idx
                fill_state[1] = slot + 1

            ctc = [ctc_pool.tile([P, CH], bf16, name=f"ctc{m}",
                                 tag=f"ctc{m}")
                   for m in range(NMT)]

            def emit_norm_dve(pcq, ctq):
                # denominators live at col 64 of each qb block ->
                # per-partition scale, no PE broadcast needed.
                rec4 = rec_pool.tile([P, 4], bf16)
                pcq3 = pcq[:].rearrange("p (qb c) -> p qb c", c=HD + 1)
                rec3 = rec4[:].rearrange("p (b o) -> p b o", o=1)
                with nc.allow_low_precision(
                    reason="1/l rounded to bf16 scale"
                ):
                    nc.vector.reciprocal(rec3, pcq3[:, :, HD : HD + 1])
                nc.vector.tensor_mul(
                    ctq[:].rearrange("p (qb c) -> p qb c", c=HD),
                    pcq3[:, :, 0:HD],
                    rec3.broadcast_to([P, 4, HD]),
                )

            def emit_norm_tpose(ctq, mt, hrow):
                # [q, hd] -> [hd, q] for the f-major Wo projection
                for qb in range(4):
                    ptq = ps_s_pool.tile([HD, P], bf16, tag="ps")
                    nc.tensor.transpose(
                        ptq[:], ctq[:, qb * HD : (qb + 1) * HD], ident[:]
                    )
                    nc.vector.tensor_copy(
                        ctc[mt][hrow : hrow + HD, qb * P : (qb + 1) * P],
                        ptq[:],
                    )

            pending_tpose = None
            for h in range(HL):
                mt = h // 2
                hrow = (h % 2) * HD
                pcq = pcm_pool.tile([P, 4 * (HD + 1)], f32, tag="pcm")
                ctq = ctq_pool.tile([P, 4 * HD], bf16)
                pairs = list(range(0, nka_q, 2))
                ps2s = {}
                pt2s = {}

                def emit_scores(kt2):
                    diag = kt2 >= 4 * qc
                    ps2 = ps_s_pool.tile([P, 2 * CH], f32, tag="ps")
                    pt2 = ptile_pool.tile([P, 2 * CH], bf16)
                    ps2s[kt2] = ps2
                    pt2s[kt2] = pt2
                    for u in range(2):
                        kt = kt2 + u
                        lo = (kt - 4 * qc) * P if diag else 0
                        nc.tensor.matmul(
                            ps2[:, u * CH + lo : (u + 1) * CH],
                            KT[bk][mt][hrow : hrow + HD,
                                       kt * P : (kt + 1) * P],
                            QT[bk][mt][hrow : hrow + HD,
                                       qc * CH + lo : (qc + 1) * CH],
                            start=True,
                            stop=True,
                        )
                    if not diag:
                        nc.scalar.activation(
                            pt2[:], ps2[:], AF.Exp, scale=0.125
                        )
                    else:
                        for u in range(2):
                            lo = (kt2 + u - 4 * qc) * P
                            if lo:
                                nc.gpsimd.memset(
                                    pt2[:, u * CH : u * CH + lo], 0.0
                                )
                            nc.scalar.activation(
                                pt2[:, u * CH + lo : (u + 1) * CH],
                                ps2[:, u * CH + lo : (u + 1) * CH],
                                AF.Exp, scale=0.125,
                            )
                            nc.vector.tensor_mul(
                                pt2[:, u * CH + lo : u * CH + lo + P],
                                pt2[:, u * CH + lo : u * CH + lo + P],
                                tri[:],
                            )

                def emit_ctx(kt2):
                    pt2 = pt2s.pop(kt2)
                    ps2s.pop(kt2)
                    for u in range(2):
                        kt = kt2 + u
                        j = kt - 4 * qc  # diag block index if >= 0
                        for qb in range(max(0, j), 4):
                            # start=True clears has_written for the WHOLE
                            # bank, so only the very first matmul into the
                            # pcq bank may set it; later qb first-writes
                            # overwrite-where-clear per element.
                            nc.tensor.matmul(
                                pcq[:, qb * (HD + 1)
                                    : qb * (HD + 1) + HD + 1],
                                pt2[:, u * CH + qb * P
                                    : u * CH + (qb + 1) * P],
                                VE4[bk][:, h, kt, :],
                                start=(kt == 0 and qb == max(0, j)),
                                stop=(kt == min(nka_q - 1, 4 * qc + qb)),
                            )

                # software-pipelined emission: scores of pair i+1 land on
                # the PE queue before ctx of pair i, so PE streams while
                # ScalarE runs exp; the previous head's ctx transposes
                # are emitted one pair in.
                emit_scores(pairs[0])
                for i, kt2 in enumerate(pairs):
                    if i + 1 < len(pairs):
                        emit_scores(pairs[i + 1])
                    emit_ctx(kt2)
                    if (i == min(1, len(pairs) - 1)
                            and pending_tpose is not None):
                        emit_norm_tpose(*pending_tpose)
                        pending_tpose = None
                    drain_fill()
                # the DVE part runs now (frees pcq for the pool); only
                # the PE transposes are deferred.
                emit_norm_dve(pcq, ctq)
                pending_tpose = (ctq, mt, hrow)
            emit_norm_tpose(*pending_tpose)
            while fill_state[0] < len(fill):
                fill[fill_state[0]]()
                fill_state[0] += 1
            return ctc

        # ---- the flattened rep x chunk stream -------------------------
        seq = [(r, t) for r in range(reps) for t in range(NCH)]
        xt_t = dma_xt(0)
        for g in proj_groups(0, 0, xt_t):
            g()
        pending_wo = []
        for idx, (r, tch) in enumerate(seq):
            fill = list(pending_wo)
            pending_wo = []
            if idx + 1 < len(seq):
                nr, nt = seq[idx + 1]
                xt_t = dma_xt(nt)
                fill += proj_groups(nr % nbank, nt, xt_t)
            ctc = attention_chunk(r % nbank, tch, fill)
            pending_wo = wo_groups(tch, ctc)
        for g in pending_wo:
            g()

    nc.compile()
    return nc


def _get_nc(reps=1):
    key = f"nc{reps}"
    if key not in _NC_CACHE:
        _NC_CACHE[key] = _build_nc(reps)
    return _NC_CACHE[key]


def _make_in_maps(inputs):
    x = np.asarray(inputs["x"], dtype=np.float32)
    W_q = np.asarray(inputs["W_q"], dtype=np.float32)
    W_k = np.asarray(inputs["W_k"], dtype=np.float32)
    W_v = np.asarray(inputs["W_v"], dtype=np.float32)
    W_o = np.asarray(inputs["W_o"], dtype=np.float32)
    in_maps = []
    for c in range(8):
        b = c % 4
        hh = c // 4
        cols = slice(hh * F, (hh + 1) * F)
        in_maps.append(
            {
                "xt": np.ascontiguousarray(x[b].T).astype(BF16),
                "wq": np.ascontiguousarray(W_q[:, cols]).astype(BF16),
                "wk": np.ascontiguousarray(W_k[:, cols]).astype(BF16),
                "wv": np.ascontiguousarray(W_v[:, cols]).astype(BF16),
                "wo": np.ascontiguousarray(W_o[cols, :]).astype(BF16),
            }
        )
    return in_maps


def kernel(x, W_q, W_k, W_v, W_o, b_o):
    from concourse.bass_utils import run_bass_kernel_spmd

    b_o = np.asarray(b_o, dtype=np.float32)
    nc = _get_nc()
    in_maps = _make_in_maps(
        {"x": x, "W_q": W_q, "W_k": W_k, "W_v": W_v, "W_o": W_o}
    )
    res = run_bass_kernel_spmd(nc, in_maps, core_ids=list(range(8)))

    full = np.empty((B, S, D), dtype=np.float32)
    for b in range(B):
        full[b] = res.results[b]["out"] + res.results[b + 4]["out"] + b_o
    return full


# revision 48
# speedup vs baseline: 5.3863x; 1.3795x over previous
"""Causal MHA kernel for 8 TRN2 NeuronCores.

Problem: x[4,2048,1024], 16 heads, hd=64, causal softmax attention, f32.

Sharding: core c handles batch c%4 and head-half c//4 (8 heads).
Each core computes its 8 heads' attention plus the row-slice of the
output projection; the host sums the two partials per batch (the
all-reduce of the row-parallel W_o split) and adds b_o.

v5: bf16 operands everywhere (PSUM accumulation stays f32; operand
quantization ~0.5% rel err, gate 2e-2). Weights are DMA'd once and
stay resident in SBUF.

Schedule: one software-pipelined stream over rep x token-chunk. During
attention for query chunk t (paced by ScalarE exp throughput), the PE
work of the NEXT chunk's Q/K/V projections and the PREVIOUS chunk's
output projection is drained in at key-pair granularity, so the
in-order PE queue always has independent work while ScalarE drains.
The chunk stream crosses rep boundaries; Q^T/K^T/V^T live in two
SBUF bank sets (rep parity) so the next rep's projections can overlap
the previous rep's last attention chunk without WAR stalls. ScalarE
runs ONLY exp; every PSUM->SBUF copy is on the Vector engine; memsets
and DMA dispatch are on GpSimd/SP.

Attention uses a flipped ctx product: probs tiles [keys, q] are the
stationary operand and V_ext [keys, hd+1] the moving one, producing
ctx in [q, hd] orientation with full PE output-column utilization
(half the streaming cycles of the [hd+1, q] orientation) and natural
causal skipping of all-masked 128-blocks. The trailing ones-column of
V_ext makes PSUM col 64 of each q-block the softmax denominator, so
normalization is a per-partition reciprocal + scale on DVE (no PE
broadcast). A PE transpose per q-block returns ctx to the f-major
layout the W_o projection needs. Within a head, score matmuls of
key-pair i+1 are emitted before the ctx matmuls of pair i; a head's
ctx transposes are deferred into the next head's stream. Diagonal
128x512 score tiles are trimmed: memset-zero the below-diagonal
columns, exp only the valid range, one [128,128] triangular mask.

Device-side layout: everything transposed. Host ships x[b].T so the
contraction dim (D) lands on SBUF partitions.
"""

import numpy as np
import ml_dtypes

BF16 = ml_dtypes.bfloat16

B, S, D, H, HD = 4, 2048, 1024, 16, 64
HL = 8            # heads per core
F = HL * HD       # 512 local head features
P = 128
CH = 512          # free-dim chunk for matmuls
NKT = D // P      # 8 contraction tiles for projections
NMT = F // P      # 4 head-pair tiles
NCH = S // CH     # 4 token chunks
NKA = S // P      # 16 attention key tiles

_NC_CACHE = {}


def _build_nc(reps=1):
    from contextlib import ExitStack

    import concourse.bass as bass
    import concourse.tile as tile
    from concourse import bacc, mybir
    from concourse.masks import make_identity

    f32 = mybir.dt.float32
    bf16 = mybir.dt.bfloat16
    AF = mybir.ActivationFunctionType
    ALU = mybir.AluOpType

    nc = bacc.Bacc("TRN2", target_bir_lowering=False)
    xt_d = nc.declare_dram_parameter("xt", [D, S], bf16, isOutput=False)
    wq_d = nc.declare_dram_parameter("wq", [D, F], bf16, isOutput=False)
    wk_d = nc.declare_dram_parameter("wk", [D, F], bf16, isOutput=False)
    wv_d = nc.declare_dram_parameter("wv", [D, F], bf16, isOutput=False)
    wo_d = nc.declare_dram_parameter("wo", [F, D], bf16, isOutput=False)
    out_d = nc.declare_dram_parameter("out", [S, D], f32, isOutput=True)
    w_by_name = {"q": wq_d, "k": wk_d, "v": wv_d}

    nbank = 2 if reps > 1 else 1

    with tile.TileContext(nc) as tc, ExitStack() as ctx:
        const_pool = ctx.enter_context(tc.tile_pool(name="const", bufs=1))
        qt_pool = ctx.enter_context(tc.tile_pool(name="qt", bufs=1))
        ve_pool = ctx.enter_context(tc.tile_pool(name="ve", bufs=1))
        wo_pool = ctx.enter_context(tc.tile_pool(name="wo", bufs=1))
        ws_pool = ctx.enter_context(tc.tile_pool(name="ws", bufs=1))
        xt_pool = ctx.enter_context(tc.tile_pool(name="xt", bufs=2))
        vs_pool = ctx.enter_context(tc.tile_pool(name="vstage", bufs=2))
        ptile_pool = ctx.enter_context(tc.tile_pool(name="ptile", bufs=3))
        ctc_pool = ctx.enter_context(tc.tile_pool(name="ctc", bufs=2))
        rec_pool = ctx.enter_context(tc.tile_pool(name="rec", bufs=2))
        ctq_pool = ctx.enter_context(tc.tile_pool(name="ctq", bufs=2))
        osb_pool = ctx.enter_context(tc.tile_pool(name="osb", bufs=2))
        pp_pool = ctx.enter_context(
            tc.tile_pool(name="pp", bufs=2, space="PSUM"))
        ps_s_pool = ctx.enter_context(
            tc.tile_pool(name="ps_s", bufs=2, space="PSUM"))
        pcm_pool = ctx.enter_context(
            tc.tile_pool(name="pcm", bufs=2, space="PSUM"))

        ident = const_pool.tile([P, P], bf16)
        make_identity(nc, ident[:])
        onesf = const_pool.tile([P, 1], bf16)
        nc.vector.memset(onesf[:], 1.0)

        # double-banked Q^T/K^T/V_ext (rep parity) so rep r+1's
        # projections overlap rep r's last attention chunk
        QT, KT, VE4 = [], [], []
        for bk in range(nbank):
            QT.append([qt_pool.tile([P, S], bf16, name=f"qt{bk}_{m}",
                                    tag=f"qt{bk}_{m}")
                       for m in range(NMT)])
            KT.append([qt_pool.tile([P, S], bf16, name=f"kt{bk}_{m}",
                                    tag=f"kt{bk}_{m}")
                       for m in range(NMT)])
            # V_ext: per (head, key-tile) a [128, 65] stationary block;
            # col 64 stays 1.0 (projection copies only touch cols 0..63).
            VE = ve_pool.tile([P, HL * NKA * 65], bf16, name=f"ve{bk}",
                              tag=f"ve{bk}")
            nc.vector.tensor_copy(
                VE[:].rearrange("p (b c) -> p b c", c=65)[:, :, 64:65],
                onesf[:].broadcast_to([P, HL * NKA, 1]),
            )
            VE4.append(VE[:].rearrange("p (h ka c) -> p h ka c",
                                       h=HL, c=65))

        # single [128,128] lower-triangular keep-mask: tri[k,c]=1 iff c>=k
        tri = const_pool.tile([P, P], bf16)
        nc.vector.memset(tri[:], 1.0)
        nc.gpsimd.affine_select(
            out=tri[:],
            in_=tri[:],
            compare_op=ALU.is_ge,
            fill=0.0,
            base=0,
            pattern=[[1, P]],
            channel_multiplier=-1,
        )

        # resident weights, DMA'd in per-128-row slices from the otherwise
        # idle SP sequencer so Pool isn't a dispatch bottleneck.
        WS = {}
        for wname in ("v", "k", "q"):
            ws = ws_pool.tile([P, NKT * F], bf16, name=f"ws_{wname}",
                              tag=f"ws_{wname}")
            for kt in range(NKT):
                nc.sync.dma_start(
                    ws[:, kt * F : (kt + 1) * F],
                    w_by_name[wname][kt * P : (kt + 1) * P, :],
                )
            WS[wname] = ws
        WO = wo_pool.tile([P, NMT * D], bf16)
        for ft in range(NMT):
            nc.sync.dma_start(
                WO[:, ft * D : (ft + 1) * D],
                wo_d[ft * P : (ft + 1) * P, :],
            )

        def dma_xt(tch):
            xt_t = xt_pool.tile([P, NKT * CH], bf16)
            for kt in range(NKT):
                # Pool queue: parallel with SP's weight DMAs
                nc.gpsimd.dma_start(
                    xt_t[:, kt * CH : (kt + 1) * CH],
                    xt_d[kt * P : (kt + 1) * P,
                         tch * CH : (tch + 1) * CH],
                )
            return xt_t

        def proj_half(bk, tch, xt_t, wname, mt, half, state):
            # split per-mt projection into two ~850ns fill units sharing
            # one PSUM accumulator
            ws = WS[wname]
            k0 = half * (NKT // 2)
            if half == 0:
                state["pp"] = pp_pool.tile([P, CH], f32, name="pp",
                                           tag="pp")
            pp = state["pp"]
            for kt in range(k0, k0 + NKT // 2):
                nc.tensor.matmul(
                    pp[:],
                    ws[:, kt * F + mt * P : kt * F + (mt + 1) * P],
                    xt_t[:, kt * CH : (kt + 1) * CH],
                    start=(kt == 0),
                    stop=(kt == NKT - 1),
                )
            if half == 0:
                return
            if wname == "q":
                nc.vector.tensor_copy(
                    QT[bk][mt][:, tch * CH : (tch + 1) * CH], pp[:]
                )
            elif wname == "k":
                nc.vector.tensor_copy(
                    KT[bk][mt][:, tch * CH : (tch + 1) * CH], pp[:]
                )
            else:
                vs = vs_pool.tile([P, CH], bf16)
                nc.vector.tensor_copy(vs[:], pp[:])
                for j in range(CH // P):
                    ka = tch * (CH // P) + j
                    ptp = ps_s_pool.tile([P, P], bf16, tag="ps")
                    nc.tensor.transpose(
                        ptp[:], vs[:, j * P : (j + 1) * P], ident[:]
                    )
                    # both heads' 64-col halves in one copy
                    nc.vector.tensor_copy(
                        VE4[bk][:, 2 * mt : 2 * mt + 2, ka, 0:HD],
                        ptp[:].rearrange(
                            "p (hh c) -> p hh c", hh=2
                        )[:, :, 0:HD],
                    )

        def proj_groups(bk, tch, xt_t):
            out = []
            for w in ("v", "k", "q"):
                for m in range(NMT):
                    st = {}
                    for hf in range(2):
                        out.append(
                            lambda w=w, m=m, hf=hf, st=st:
                                proj_half(bk, tch, xt_t, w, m, hf, st)
                        )
            return out

        def wo_part(qc, ctc, tt4, ncol, state):
            if ncol == 0:
                state["osb"] = osb_pool.tile([P, D], f32, name="osb")
            osb = state["osb"]
            # pp pool, not pcm: pcq buffers are released by the (slower)
            # reciprocal+scale chain, pp buffers by a plain copy
            po = pp_pool.tile([P, CH], f32, name="po", tag="pp")
            for ft in range(NMT):
                nc.tensor.matmul(
                    po[:],
                    ctc[ft][:, tt4 * P : (tt4 + 1) * P],
                    WO[:, ft * D + ncol * CH : ft * D + (ncol + 1) * CH],
                    start=(ft == 0),
                    stop=(ft == NMT - 1),
                )
            nc.vector.tensor_copy(
                osb[:, ncol * CH : (ncol + 1) * CH], po[:]
            )
            if ncol == D // CH - 1:
                r0 = qc * CH + tt4 * P
                nc.gpsimd.dma_start(out_d[r0 : r0 + P, :], osb[:])

        def wo_groups(qc, ctc):
            out = []
            for t in range(CH // P):
                st = {}
                for ncol in range(D // CH):
                    out.append(
                        lambda t=t, ncol=ncol, st=st:
                            wo_part(qc, ctc, t, ncol, st)
                    )
            return out

        def attention_chunk(bk, qc, fill, carry_tpose=None):
            """Attention for query chunk qc reading bank bk, draining
            `fill` (list of emission callables) at key-pair granularity.
            `carry_tpose` is the previous chunk's last-head ctx transpose
            (emitted here, where its inputs are long ready). Returns
            (ctc, carry) where carry finishes this chunk's last head."""
            nka_q = 4 * qc + 4  # causal: key tiles 0..nka_q-1
            total_slots = HL * (nka_q // 2)
            fill_state = [0, 0]  # [next fill idx, slot counter]

            def drain_fill():
                idx, slot = fill_state
                while (idx < len(fill)
                       and idx * total_slots <= slot * len(fill)):
                    fill[idx]()
                    idx += 1
                fill_state[0] = idx
                fill_state[1] = slot + 1

            ctc = [ctc_pool.tile([P, CH], bf16, name=f"ctc{m}",
                                 tag=f"ctc{m}")
                   for m in range(NMT)]

            def emit_norm_dve(pcq, ctq):
                # denominators live at col 64 of each qb block ->
                # per-partition scale, no PE broadcast needed.
                rec4 = rec_pool.tile([P, 4], bf16)
                pcq3 = pcq[:].rearrange("p (qb c) -> p qb c", c=HD + 1)
                rec3 = rec4[:].rearrange("p (b o) -> p b o", o=1)
                with nc.allow_low_precision(
                    reason="1/l rounded to bf16 scale"
                ):
                    nc.vector.reciprocal(rec3, pcq3[:, :, HD : HD + 1])
                nc.vector.tensor_mul(
                    ctq[:].rearrange("p (qb c) -> p qb c", c=HD),
                    pcq3[:, :, 0:HD],
                    rec3.broadcast_to([P, 4, HD]),
                )

            def emit_norm_tpose(ctq, mt, hrow):
                # [q, hd] -> [hd, q] for the f-major Wo projection
                for qb in range(4):
                    ptq = ps_s_pool.tile([HD, P], bf16, tag="ps")
                    nc.tensor.transpose(
                        ptq[:], ctq[:, qb * HD : (qb + 1) * HD], ident[:]
                    )
                    nc.vector.tensor_copy(
                        ctc[mt][hrow : hrow + HD, qb * P : (qb + 1) * P],
                        ptq[:],
                    )

            if carry_tpose is not None:
                carry_tpose()
            pending_tpose = None
            for h in range(HL):
                mt = h // 2
                hrow = (h % 2) * HD
                pcq = pcm_pool.tile([P, 4 * (HD + 1)], f32, tag="pcm")
                ctq = ctq_pool.tile([P, 4 * HD], bf16)
                pairs = list(range(0, nka_q, 2))
                ps2s = {}
                pt2s = {}

                def emit_scores(kt2):
                    diag = kt2 >= 4 * qc
                    ps2 = ps_s_pool.tile([P, 2 * CH], f32, tag="ps")
                    pt2 = ptile_pool.tile([P, 2 * CH], bf16)
                    ps2s[kt2] = ps2
                    pt2s[kt2] = pt2
                    for u in range(2):
                        kt = kt2 + u
                        lo = (kt - 4 * qc) * P if diag else 0
                        nc.tensor.matmul(
                            ps2[:, u * CH + lo : (u + 1) * CH],
                            KT[bk][mt][hrow : hrow + HD,
                                       kt * P : (kt + 1) * P],
                            QT[bk][mt][hrow : hrow + HD,
                                       qc * CH + lo : (qc + 1) * CH],
                            start=True,
                            stop=True,
                        )
                    if not diag:
                        nc.scalar.activation(
                            pt2[:], ps2[:], AF.Exp, scale=0.125
                        )
                    else:
                        # flipped ctx only reads q-blocks >= j, so the
                        # below-diagonal columns need no zeroing at all
                        for u in range(2):
                            lo = (kt2 + u - 4 * qc) * P
                            nc.scalar.activation(
                                pt2[:, u * CH + lo : (u + 1) * CH],
                                ps2[:, u * CH + lo : (u + 1) * CH],
                                AF.Exp, scale=0.125,
                            )
                            nc.vector.tensor_mul(
                                pt2[:, u * CH + lo : u * CH + lo + P],
                                pt2[:, u * CH + lo : u * CH + lo + P],
                                tri[:],
                            )

                def emit_ctx(kt2):
                    pt2 = pt2s.pop(kt2)
                    ps2s.pop(kt2)
                    for u in range(2):
                        kt = kt2 + u
                        j = kt - 4 * qc  # diag block index if >= 0
                        # masked (triangular) q-block last: its DVE mask
                        # finishes while the other blocks' ctx runs
                        if j >= 0:
                            qb_order = list(range(j + 1, 4)) + [j]
                        else:
                            qb_order = list(range(4))
                        for qb in qb_order:
                            # start=True clears has_written for the WHOLE
                            # bank, so only the very first matmul into the
                            # pcq bank may set it; later qb first-writes
                            # overwrite-where-clear per element.
                            nc.tensor.matmul(
                                pcq[:, qb * (HD + 1)
                                    : qb * (HD + 1) + HD + 1],
                                pt2[:, u * CH + qb * P
                                    : u * CH + (qb + 1) * P],
                                VE4[bk][:, h, kt, :],
                                start=(kt == 0 and qb == qb_order[0]),
                                stop=(kt == min(nka_q - 1, 4 * qc + qb)),
                            )

                # software-pipelined emission: scores of pair i+1 land on
                # the PE queue before ctx of pair i, so PE streams while
                # ScalarE runs exp; the previous head's ctx transposes
                # are emitted one pair in.
                emit_scores(pairs[0])
                for i, kt2 in enumerate(pairs):
                    if i + 1 < len(pairs):
                        emit_scores(pairs[i + 1])
                    # fill lands BEFORE this pair's ctx so the PE has
                    # queued work while the pair's exp drains
                    drain_fill()
                    emit_ctx(kt2)
                    if (i == min(1, len(pairs) - 1)
                            and pending_tpose is not None):
                        emit_norm_tpose(*pending_tpose)
                        pending_tpose = None
                # the DVE part runs now (frees pcq for the pool); only
                # the PE transposes are deferred.
                emit_norm_dve(pcq, ctq)
                pending_tpose = (ctq, mt, hrow)
            while fill_state[0] < len(fill):
                fill[fill_state[0]]()
                fill_state[0] += 1
            args = pending_tpose
            carry = lambda a=args: emit_norm_tpose(*a)
            return ctc, carry

        # ---- the flattened rep x chunk stream -------------------------
        seq = [(r, t) for r in range(reps) for t in range(NCH)]
        xt_t = dma_xt(0)
        for g in proj_groups(0, 0, xt_t):
            g()
        pending_wo = []
        carry = None
        for idx, (r, tch) in enumerate(seq):
            fill = list(pending_wo)
            pending_wo = []
            if idx + 1 < len(seq):
                nr, nt = seq[idx + 1]
                xt_t = dma_xt(nt)
                fill += proj_groups(nr % nbank, nt, xt_t)
            ctc, carry = attention_chunk(r % nbank, tch, fill, carry)
            pending_wo = wo_groups(tch, ctc)
        carry()
        for g in pending_wo:
            g()

    nc.compile()
    return nc


def _get_nc(reps=1):
    key = f"nc{reps}"
    if key not in _NC_CACHE:
        _NC_CACHE[key] = _build_nc(reps)
    return _NC_CACHE[key]


def _make_in_maps(inputs):
    x = np.asarray(inputs["x"], dtype=np.float32)
    W_q = np.asarray(inputs["W_q"], dtype=np.float32)
    W_k = np.asarray(inputs["W_k"], dtype=np.float32)
    W_v = np.asarray(inputs["W_v"], dtype=np.float32)
    W_o = np.asarray(inputs["W_o"], dtype=np.float32)
    in_maps = []
    for c in range(8):
        b = c % 4
        hh = c // 4
        cols = slice(hh * F, (hh + 1) * F)
        in_maps.append(
            {
                "xt": np.ascontiguousarray(x[b].T).astype(BF16),
                "wq": np.ascontiguousarray(W_q[:, cols]).astype(BF16),
                "wk": np.ascontiguousarray(W_k[:, cols]).astype(BF16),
                "wv": np.ascontiguousarray(W_v[:, cols]).astype(BF16),
                "wo": np.ascontiguousarray(W_o[cols, :]).astype(BF16),
            }
        )
    return in_maps


def kernel(x, W_q, W_k, W_v, W_o, b_o):
    from concourse.bass_utils import run_bass_kernel_spmd

    b_o = np.asarray(b_o, dtype=np.float32)
    nc = _get_nc()
    in_maps = _make_in_maps(
        {"x": x, "W_q": W_q, "W_k": W_k, "W_v": W_v, "W_o": W_o}
    )
    res = run_bass_kernel_spmd(nc, in_maps, core_ids=list(range(8)))

    full = np.empty((B, S, D), dtype=np.float32)
    for b in range(B):
        full[b] = res.results[b]["out"] + res.results[b + 4]["out"] + b_o
    return full
